# revision 20
# baseline (speedup 1.0000x reference)
"""GCN (2-layer) Trainium2 kernel over 8 NeuronCores — v4.

Per core (dst-shard = 6250 nodes = 49 tiles of 128):
- head: Tsh = dinv * (x @ W1) bf16 rows [node, 128] (64 feats + 64 pad so
  gather rows are 256B). Table row-halves (shard tiles 0:25 / 25:49) get
  separate AllGathers per layer so half-B's AG overlaps half-A compute;
  layer-2 AGs fire from inside layer-1 tails (MIDAG).
- aggregation: edges sorted per (dst tile t, half h) with uniform slot
  count S_t[t,h] (cross-core max, padded to mult of 16; pad slots gather
  row 0 and carry dst-lane 255). Gathers are CHUNKED: one SWDGE
  dma_gather per (chunk of 4 tiles, h) on 4 round-robin SWDGE queues
  (4 Q7 pairs in parallel). Scatter one-hots are built on-device per
  (piece, tile) column by DVE is_equal(iota, dl) (tensor_tensor = no
  SBUF-port contention with SWDGE); pieces may straddle 2 tiles (two
  columns). psum[t] += S_col^T @ G_piece.
- PSUM: 8 accumulators per 2KB bank ([128,512] f32 bank tiles, slices of
  64 cols); start=True only on a bank's first matmul (whole-bank reset);
  tails run per-bank AFTER the bank's last matmul (PE-write + DVE-read of
  one bank is a fatal HW collision).
- tails on the Activation engine where possible (tensor_scalar/copy on
  DVE block SWDGE descriptor gen via the shared SBUF port).
"""

import numpy as np

N_NODES = 50000
CORES = 8
SH = 6250          # owned nodes per core
SHP = 6272         # padded shard rows (49*128)
NT = 49            # dst tiles per core
TA = 25            # tiles in table half A
HA = TA * 128      # 3200 rows per core in half A
HB = SHP - HA      # 3072 rows per core in half B
ROWS_A = CORES * HA   # 25600
ROWS_B = CORES * HB   # 24576
F0, F1, F2 = 96, 64, 16
FP = 128           # padded feature width (bf16 row = 256B)
BLK = 128
CHT = 4            # tiles per gather chunk
NCH = (NT + CHT - 1) // CHT   # 13 chunks
PAD_LANE = 255.0


def _chunk_tiles(c):
    return range(c * CHT, min((c + 1) * CHT, NT))


def _columns(S, c, h):
    """Uniform (piece, tile, [tile slot range within piece]) column list for
    chunk c, half h, from the uniform slot counts S[t,h]."""
    tiles = list(_chunk_tiles(c))
    bounds = np.cumsum([0] + [int(S[t, h]) for t in tiles])
    L = int(bounds[-1])
    cols = []
    npieces = (L + BLK - 1) // BLK
    for p in range(npieces):
        lo, hi = p * BLK, min((p + 1) * BLK, L)
        for ti, t in enumerate(tiles):
            a, b = max(lo, bounds[ti]), min(hi, bounds[ti + 1])
            if a < b:
                cols.append((p, t, a - lo, b - lo))
    return cols, L, npieces


def host_prep(x, edge_index, W1, b1, W2, b2):
    import ml_dtypes
    bf16 = ml_dtypes.bfloat16

    src = np.asarray(edge_index[0], dtype=np.int64)
    dst = np.asarray(edge_index[1], dtype=np.int64)
    deg_full = np.bincount(dst, minlength=N_NODES).astype(np.float32) + 1.0

    own = src // SH
    r = src - own * SH
    h_of = (r // 128 >= TA).astype(np.int64)
    lrow_of = np.where(h_of == 0, HA * own + r, HB * own + (r - HA))

    order = np.argsort(dst, kind="stable")
    s_sorted, d_sorted = src[order], dst[order]
    h_sorted, lrow_sorted = h_of[order], lrow_of[order]
    bounds = np.searchsorted(d_sorted, np.arange(0, N_NODES + 1, SH))
    runs = [[[None] * 2 for _ in range(NT)] for _ in range(CORES)]
    counts = np.zeros((CORES, NT, 2), dtype=np.int64)
    for i in range(CORES):
        sl = slice(bounds[i], bounds[i + 1])
        ed = d_sorted[sl] - SH * i
        hh = h_sorted[sl]
        lr = lrow_sorted[sl]
        tile_id = ed // BLK
        dl = ed - tile_id * BLK
        key = tile_id * 2 + hh
        o = np.lexsort((lr, key))
        key_s, lr_s, dl_s = key[o], lr[o], dl[o]
        kb = np.searchsorted(key_s, np.arange(NT * 2 + 1))
        for t in range(NT):
            for h2 in (0, 1):
                a, b = kb[t * 2 + h2], kb[t * 2 + h2 + 1]
                runs[i][t][h2] = (lr_s[a:b], dl_s[a:b])
                counts[i, t, h2] = b - a

    # uniform per-(tile,half) slot counts, mult of 16
    S = ((counts.max(axis=0) + 15) // 16 * 16).astype(np.int64)  # [NT,2]
    S = np.maximum(S, 16)

    # column schedule (uniform across cores)
    colmeta = {}   # (c,h) -> (cols, L, npieces, colbase)
    chunk_len = np.zeros((NCH, 2), np.int64)
    ncols_tot = 0
    for c in range(NCH):
        for h2 in (0, 1):
            cols, L, npieces = _columns(S, c, h2)
            colmeta[(c, h2)] = (cols, L, npieces, ncols_tot)
            chunk_len[c, h2] = L
            ncols_tot += len(cols)
    half_len = [int(chunk_len[:, h2].sum()) for h2 in (0, 1)]

    data = []
    for i in range(CORES):
        idx_streams = [[], []]
        dl_blob = np.full((BLK, ncols_tot), PAD_LANE, dtype=np.float32)
        for c in range(NCH):
            for h2 in (0, 1):
                cols, L, npieces, colbase = colmeta[(c, h2)]
                stream = np.zeros(L, np.int64)
                lanes = np.full(L, PAD_LANE, np.float32)
                off = 0
                for t in _chunk_tiles(c):
                    lr, dl = runs[i][t][h2]
                    n = len(lr)
                    stream[off:off + n] = lr
                    lanes[off:off + n] = dl
                    off += int(S[t, h2])
                idx_streams[h2].append(stream)
                for j, (p, t, a, b) in enumerate(cols):
                    seg = lanes[p * BLK + a:p * BLK + b]
                    col = np.full(BLK, PAD_LANE, np.float32)
                    col[a:b] = seg
                    dl_blob[:, colbase + j] = col
        planes = []
        for h2 in (0, 1):
            si = np.concatenate(idx_streams[h2])
            assert len(si) == half_len[h2]
            pl = si.reshape(-1, 16).T.astype(np.int16)
            planes.append(np.tile(pl, (8, 1)))

        degp = np.ones((BLK, NT), np.float32)
        dshard = deg_full[SH * i:SH * (i + 1)]
        dp = np.concatenate([dshard, np.ones(SHP - SH, np.float32)])
        degp[:, :] = dp.reshape(NT, BLK).T

        xs = np.zeros((F0, SHP), np.float32)
        xs[:, :SH] = np.asarray(x[SH * i:SH * (i + 1)], np.float32).T
        data.append(dict(
            xT=np.ascontiguousarray(xs.astype(bf16)),
            idx0=np.ascontiguousarray(planes[0]),
            idx1=np.ascontiguousarray(planes[1]),
            dl=np.ascontiguousarray(dl_blob.astype(bf16)),
            deg=np.ascontiguousarray(degp),
        ))

    consts = dict(
        W1=np.asarray(W1, np.float32).astype(bf16),
        W2=np.asarray(W2, np.float32).astype(bf16),
        b1b=np.tile(np.asarray(b1, np.float32), (BLK, 1)),
        b2b=np.tile(np.asarray(b2, np.float32), (BLK, 1)),
        identb=np.eye(BLK, dtype=np.float32).astype(bf16),
        identf=np.eye(BLK, dtype=np.float32),
        iota=np.ascontiguousarray(
            np.tile(np.arange(BLK, dtype=np.float32), (BLK, 1)).astype(bf16)),
    )
    meta = dict(S=S, half_len=half_len, colmeta=colmeta,
                chunk_len=chunk_len, ncols_tot=ncols_tot)
    return data, consts, meta


def numpy_device_sim(data, consts, meta):
    S = meta["S"]
    colmeta = meta["colmeta"]
    dinvs, tables = [], []
    for i in range(CORES):
        d = data[i]
        dinv = 1.0 / np.sqrt(d["deg"])
        dinvs.append(dinv)
        hmat = d["xT"].astype(np.float32).T @ consts["W1"].astype(np.float32)
        hs = hmat.reshape(NT, BLK, F1) * dinv.T[:, :, None]
        tables.append(hs.reshape(SHP, F1))

    def make_halves(tbls):
        A = np.concatenate([t[:HA] for t in tbls], 0)
        Bt = np.concatenate([t[HA:] for t in tbls], 0)
        return [A, Bt]

    def layer(halves, i, d, own):
        agg = np.zeros((NT, BLK, F1), np.float32)
        for t in range(NT):
            agg[t] = own[t]
        streams = [(d["idx0"] if h2 == 0 else d["idx1"])[:16].T.reshape(-1)
                   for h2 in (0, 1)]
        dlb = d["dl"].astype(np.float32)
        soff = [0, 0]
        for c in range(NCH):
            for h2 in (0, 1):
                cols, L, npieces, colbase = colmeta[(c, h2)]
                st = streams[h2][soff[h2]:soff[h2] + L].astype(np.int64)
                G = np.zeros((npieces * BLK, F1), np.float32)
                G[:L] = halves[h2][st]
                for j, (p, t, a, b) in enumerate(cols):
                    dl = dlb[:, colbase + j]
                    Smat = (dl[:, None] == np.arange(BLK)[None, :]
                            ).astype(np.float32)
                    agg[t] += Smat.T @ G[p * BLK:(p + 1) * BLK]
                soff[h2] += L
        return agg

    full2 = []
    halves1 = make_halves(tables)
    for i in range(CORES):
        d = data[i]
        own = tables[i].reshape(NT, BLK, F1)
        agg = layer(halves1, i, d, own)
        dinv = dinvs[i]
        t2 = []
        for t in range(NT):
            e = np.maximum(agg[t] * dinv[:, t:t + 1] + consts["b1b"], 0.0) \
                * dinv[:, t:t + 1]
            t2.append(e)
        full2.append(np.stack(t2).reshape(SHP, F1))

    outs = []
    halves2 = make_halves(full2)
    for i in range(CORES):
        d = data[i]
        own2 = full2[i].reshape(NT, BLK, F1)
        agg = layer(halves2, i, d, own2)
        dinv = dinvs[i]
        o = np.zeros((NT, BLK, F2), np.float32)
        for t in range(NT):
            a = agg[t] * dinv[:, t:t + 1]
            z = a @ consts["W2"].astype(np.float32) + consts["b2b"]
            m = z.max(1, keepdims=True)
            ls = z - m - np.log(np.exp(z - m).sum(1, keepdims=True))
            o[t] = ls
        outs.append(o.reshape(SHP, F2))
    return np.stack(outs)


def assemble_output(outs):
    res = np.zeros((N_NODES, F2), np.float32)
    for i in range(CORES):
        res[SH * i:SH * (i + 1)] = outs[i][:SH]
    return res


def build_nc(meta):
    import concourse.bacc as bacc
    import concourse.tile as tile
    import concourse.mybir as mybir

    dt = mybir.dt
    Alu = mybir.AluOpType
    Act = mybir.ActivationFunctionType
    S = meta["S"]
    half_len = meta["half_len"]
    colmeta = meta["colmeta"]
    ncols_tot = meta["ncols_tot"]
    chunk_len = meta["chunk_len"]
    PMAX = int(max(colmeta[k][2] for k in colmeta))         # pieces per chunk
    CMAX = int(max(len(colmeta[k][0]) for k in colmeta))    # cols per chunk

    nc = bacc.Bacc(None, target_bir_lowering=False, num_swdge_queues=4)
    p_xT = nc.declare_dram_parameter("xT", [F0, SHP], dt.bfloat16, isOutput=False)
    p_idx = [nc.declare_dram_parameter(f"idx{h}", [128, half_len[h] // 16],
                                       dt.int16, isOutput=False) for h in (0, 1)]
    p_dl = nc.declare_dram_parameter("dl", [128, ncols_tot], dt.bfloat16,
                                     isOutput=False)
    p_deg = nc.declare_dram_parameter("deg", [128, NT], dt.float32, isOutput=False)
    p_W1 = nc.declare_dram_parameter("W1", [F0, F1], dt.bfloat16, isOutput=False)
    p_W2 = nc.declare_dram_parameter("W2", [F1, F2], dt.bfloat16, isOutput=False)
    p_b1 = nc.declare_dram_parameter("b1b", [128, F1], dt.float32, isOutput=False)
    p_b2 = nc.declare_dram_parameter("b2b", [128, F2], dt.float32, isOutput=False)
    p_ib = nc.declare_dram_parameter("identb", [128, 128], dt.bfloat16,
                                     isOutput=False)
    p_if = nc.declare_dram_parameter("identf", [128, 128], dt.float32,
                                     isOutput=False)
    p_iota = nc.declare_dram_parameter("iota", [128, 128], dt.bfloat16,
                                       isOutput=False)
    p_out = nc.declare_dram_parameter("out", [128, NT * F2], dt.float32,
                                      isOutput=True)

    cc_in = [[nc.dram_tensor(f"cc_in{li}{hn}", [n, FP], dt.bfloat16)
              for hn, n in (("a", HA), ("b", HB))] for li in (0, 1)]
    cc_out = [[nc.dram_tensor(f"cc_out{li}{hn}", [n, FP], dt.bfloat16,
                              addr_space="Shared")
               for hn, n in (("a", ROWS_A), ("b", ROWS_B))] for li in (0, 1)]

    with tile.TileContext(nc) as tc:
        with (
            tc.tile_pool(name="cpool", bufs=1) as cpool,
            tc.tile_pool(name="stpool", bufs=10) as stpool,
            tc.tile_pool(name="spool", bufs=5) as spool,
            tc.tile_pool(name="wpool", bufs=6) as wpool,
            tc.tile_pool(name="apool", bufs=7, space="PSUM") as apool,
            tc.tile_pool(name="xpool", bufs=1, space="PSUM") as xpool,
        ):
            xT = cpool.tile([F0, SHP], dt.bfloat16)
            nc.sync.dma_start(xT[:], p_xT[:])
            W1 = cpool.tile([F0, F1], dt.bfloat16)
            nc.sync.dma_start(W1[:], p_W1[:])
            W2 = cpool.tile([F1, F2], dt.bfloat16)
            nc.sync.dma_start(W2[:], p_W2[:])
            b1b = cpool.tile([128, F1], dt.float32)
            nc.sync.dma_start(b1b[:], p_b1[:])
            b2b = cpool.tile([128, F2], dt.float32)
            nc.sync.dma_start(b2b[:], p_b2[:])
            identb = cpool.tile([128, 128], dt.bfloat16)
            nc.sync.dma_start(identb[:], p_ib[:])
            identf = cpool.tile([128, 128], dt.float32)
            nc.sync.dma_start(identf[:], p_if[:])
            iota = cpool.tile([128, 128], dt.bfloat16)
            nc.sync.dma_start(iota[:], p_iota[:])
            degt = cpool.tile([128, NT], dt.float32)
            nc.sync.dma_start(degt[:], p_deg[:])
            dlt = cpool.tile([128, ncols_tot], dt.bfloat16)
            nc.sync.dma_start(dlt[:], p_dl[:])
            idx_sb = []
            for h in (0, 1):
                isb = cpool.tile([128, half_len[h] // 16], dt.int16,
                                 name=f"isb{h}")
                nc.sync.dma_start(isb[:], p_idx[h][:])
                idx_sb.append(isb)

            recd = cpool.tile([128, NT], dt.float32)
            nc.vector.reciprocal(recd[:], degt[:])
            dinv = cpool.tile([128, NT], dt.float32)
            nc.scalar.activation(dinv[:], recd[:], Act.Sqrt)

            Tsh = cpool.tile([128, NT * FP], dt.bfloat16)
            T2sh = cpool.tile([128, NT * FP], dt.bfloat16)
            outsh = cpool.tile([128, NT * F2], dt.float32)
            E4sh = cpool.tile([128, NT * F2], dt.float32)
            Msh = cpool.tile([128, NT], dt.float32)
            SMsh = cpool.tile([128, NT], dt.float32)

            # ---- head: Tsh = dinv * (x @ W1)   (scale on ACT engine)
            for t in range(NT):
                psh = apool.tile([128, 512], dt.float32, tag="agg",
                                 name=f"hd{t}")[:, 0:F1]
                nc.tensor.matmul(psh, xT[:, BLK * t:BLK * (t + 1)], W1[:],
                                 start=True, stop=True)
                nc.scalar.activation(Tsh[:, FP * t:FP * t + F1], psh,
                                     Act.Copy, scale=dinv[:, t:t + 1])

            def send_half(li, table_sh, hh):
                t0, t1 = (0, TA) if hh == 0 else (TA, NT)
                nc.sync.dma_start(
                    cc_in[li][hh][:].rearrange("(t p) f -> p t f", p=BLK),
                    table_sh.rearrange("p (t f) -> p t f", f=FP)[:, t0:t1, :])
                nc.gpsimd.collective_compute(
                    "AllGather", Alu.bypass,
                    ins=[cc_in[li][hh].ap().opt()],
                    outs=[cc_out[li][hh].ap().opt()],
                    replica_groups=[list(range(CORES))])

            send_half(0, Tsh, 0)
            send_half(0, Tsh, 1)

            qctr = [0]

            def do_layer(li, own, tail_fn):
                banks = [apool.tile([128, 512], dt.float32, tag="agg",
                                    name=f"bank{li}_{g}") for g in range(7)]
                paggs = [banks[t // 8][:, F1 * (t % 8):F1 * (t % 8) + F1]
                         for t in range(NT)]
                coffs = np.zeros((NCH, 2), np.int64)
                o0, o1 = 0, 0
                for c in range(NCH):
                    coffs[c, 0] = o0
                    o0 += int(chunk_len[c, 0])
                    coffs[c, 1] = o1
                    o1 += int(chunk_len[c, 1])

                done_tail = 0
                for h in (0, 1):
                    for c in range(NCH):
                        if h == 0:
                            for t in _chunk_tiles(c):
                                nc.tensor.matmul(
                                    paggs[t], identb[:],
                                    own[:, FP * t:FP * t + F1],
                                    start=(t % 8 == 0), stop=False)
                        cols, L, npieces, colbase = colmeta[(c, h)]
                        coff = int(coffs[c, h])
                        st = stpool.tile([128, PMAX, FP], dt.bfloat16,
                                         tag="st", name=f"st{li}_{h}_{c}")
                        nc.gpsimd.dma_gather(
                            st[:, :npieces, :], cc_out[li][h][:],
                            idx_sb[h][:, coff // 16:(coff + L) // 16],
                            L, L, FP, single_packet=False,
                            queue_num=qctr[0] % 4)
                        qctr[0] += 1
                        ncol = len(cols)
                        ssb = spool.tile([128, CMAX, BLK], dt.bfloat16,
                                         tag="ssb", name=f"ss{li}_{h}_{c}")
                        nc.vector.tensor_tensor(
                            out=ssb[:, :ncol, :],
                            in0=iota[:].unsqueeze(1)
                                .broadcast_to([128, ncol, 128]),
                            in1=dlt[:, colbase:colbase + ncol].unsqueeze(2)
                                .broadcast_to([128, ncol, 128]),
                            op=Alu.is_equal)
                        last_of_tile = {}
                        for j, (p, t, a, b) in enumerate(cols):
                            last_of_tile[t] = j
                        for j, (p, t, a, b) in enumerate(cols):
                            kp = min(BLK, L - p * BLK)
                            nc.tensor.matmul(
                                paggs[t], ssb[0:kp, j, :],
                                st[0:kp, p, 0:F1],
                                start=False,
                                stop=(h == 1 and last_of_tile[t] == j))
                        if h == 1:
                            # tails for fully-closed psum banks
                            tmax = list(_chunk_tiles(c))[-1]
                            while (done_tail + 8 <= tmax + 1
                                   or (tmax == NT - 1
                                       and done_tail <= tmax)):
                                t2hi = min(done_tail + 8, NT)
                                for t2 in range(done_tail, t2hi):
                                    tail_fn(t2, paggs[t2])
                                done_tail = t2hi

            # ---- layer 1
            def tail1(t, pagg):
                e1 = wpool.tile([128, F1], dt.float32, tag="e1", name=f"e1_{t}")
                nc.scalar.activation(e1[:], pagg, Act.Copy,
                                     scale=dinv[:, t:t + 1])
                e2 = wpool.tile([128, F1], dt.float32, tag="e2", name=f"e2_{t}")
                nc.vector.tensor_tensor(out=e2[:], in0=e1[:], in1=b1b[:],
                                        op=Alu.add)
                nc.scalar.activation(T2sh[:, FP * t:FP * t + F1], e2[:],
                                     Act.Relu, scale=dinv[:, t:t + 1])
                if t == TA - 1:
                    send_half(1, T2sh, 0)
                elif t == NT - 1:
                    send_half(1, T2sh, 1)

            do_layer(0, Tsh, tail1)

            # ---- layer 2
            def tail2(t, pagg):
                cp = wpool.tile([128, F1], dt.float32, tag="cp", name=f"cp_{t}")
                nc.scalar.activation(cp[:], pagg, Act.Copy,
                                     scale=dinv[:, t:t + 1])
                ptile = xpool.tile([128, 512], dt.float32, tag="pt",
                                   name=f"pt_{t}")
                ptr = ptile[0:F1, 0:128]
                nc.tensor.transpose(ptr, cp[:], identf[:])
                aggT = wpool.tile([F1, 128], dt.bfloat16, tag="at", name=f"at_{t}")
                nc.scalar.activation(aggT[:], ptr, Act.Copy)
                po = ptile[:, 128:128 + F2]
                nc.tensor.matmul(po, aggT[:], W2[:], start=True, stop=True)
                nc.vector.tensor_tensor(out=E4sh[:, F2 * t:F2 * (t + 1)],
                                        in0=po, in1=b2b[:], op=Alu.add)
                nc.vector.tensor_reduce(Msh[:, t:t + 1],
                                        E4sh[:, F2 * t:F2 * (t + 1)],
                                        axis=mybir.AxisListType.X, op=Alu.max)
                nm = wpool.tile([128, 1], dt.float32, tag="nm", name=f"nm_{t}")
                nc.scalar.activation(nm[:], Msh[:, t:t + 1], Act.Copy,
                                     scale=-1.0)
                ex = wpool.tile([128, F2], dt.float32, tag="ex", name=f"ex_{t}")
                nc.scalar.activation(ex[:], E4sh[:, F2 * t:F2 * (t + 1)],
                                     Act.Exp, bias=nm[:, 0:1],
                                     accum_out=SMsh[:, t:t + 1])

            do_layer(1, T2sh, tail2)

            lg = cpool.tile([128, NT], dt.float32)
            nc.scalar.activation(lg[:], SMsh[:], Act.Ln)
            msum = cpool.tile([128, NT], dt.float32)
            nc.vector.tensor_tensor(out=msum[:], in0=Msh[:], in1=lg[:],
                                    op=Alu.add)
            for t in range(NT):
                nc.vector.tensor_tensor(
                    out=outsh[:, F2 * t:F2 * (t + 1)],
                    in0=E4sh[:, F2 * t:F2 * (t + 1)],
                    in1=msum[:, t:t + 1].broadcast_to([128, F2]),
                    op=Alu.subtract)
            nc.sync.dma_start(p_out[:], outsh[:])

    nc.finalize()
    return nc


LAST_EXEC_NS = None


def kernel(x, edge_index, W1, b1, W2, b2):
    from concourse.bass_utils import run_bass_kernel_spmd

    x = np.asarray(x, np.float32)
    data, consts, meta = host_prep(x, np.asarray(edge_index), W1, b1, W2, b2)
    nc = build_nc(meta)
    in_maps = []
    for i in range(CORES):
        m = dict(data[i])
        m.update({k: np.ascontiguousarray(v) for k, v in consts.items()})
        in_maps.append(m)
    import os as _os
    trace = bool(int(_os.environ.get("GCN_TRACE", "0")))
    res = run_bass_kernel_spmd(nc, in_maps, core_ids=list(range(CORES)),
                               trace=trace)
    global LAST_EXEC_NS
    LAST_EXEC_NS = res.exec_time_ns
    outs = []
    for i in range(CORES):
        o = res.results[i]["out"]
        outs.append(o.reshape(128, NT, F2).transpose(1, 0, 2).reshape(SHP, F2))
    return assemble_output(np.stack(outs))


if __name__ == "__main__":
    import reference
    inputs = {k: np.asarray(v) for k, v in reference.setup_inputs().items()}
    expected = np.asarray(reference.reference(**{k: v for k, v in inputs.items()}))
    data, consts, meta = host_prep(**inputs)
    print("slots:", meta["half_len"], "ncols:", meta["ncols_tot"])
    outs = numpy_device_sim(data, consts, meta)
    got = assemble_output(outs)
    err = np.abs(got - expected)
    rel = err.max() / np.abs(expected).max()
    print(f"numpy-sim max abs err {err.max():.3e}  rel {rel:.3e}")
